# revision 32
# baseline (speedup 1.0000x reference)
"""ConvCNP encoder kernel for 8x TRN2 NeuronCores.

Math: the reference computes, for a 128x128 uniform grid g=(xs[i], ys[j]) and
n=8192 data points X (2-D) with values psi(Y) = [1, Y0, Y1]:

    Gram[g, x] = exp(-0.5*||g - X[x]||^2)
    fm = Gram @ psi                  # (G, 3); column 0 == row-sum (denominator)
    out[c, j, i] = fm[(i, j), c], with c=1,2 normalized by column 0.

The squared distance is separable over the grid axes (xs == ys == the same
128-point linspace g), and each factor splits into PE-friendly terms:

    Gram[(i,j), x] = A[i, x] * Bw[j, x]
      A[i, x]  = exp(g_i*X0[x] - g_i^2/2)
      Bw[j, x] = exp(g_j*X1[x] - g_j^2/2 - (X0[x]^2 + X1[x]^2)/2)

so, with Bc = Bw * psi_c (row-wise):  fm[(i,j), c] = sum_x Bc[j, x] * A[i, x].

Sharding: the DATA-POINT axis x across the 8 cores - 1024 points per core
(8 chunks of 128), grid replicated. Each core computes the partial
(un-normalized) feature map for the FULL grid over its x-slice; the host sums
the 8 partials and normalizes.

Engine split per core (the big idea vs the v1 kernel: NO elementwise
squared-distance pass at all - the exp arguments are tiny matmuls):

  PE   : 16 args-matmuls, one per (side, chunk): lhsT = [X block; ones]
         column-pair from the stat tile, rhs = the shared [g; -g^2/2] pair
         -> arg[x, j] = X[x]*g_j - g_j^2/2 in per-pair PSUM tiles (separate
         tiles so each exp's semaphore gates on ITS matmuls, not all 16).
         The K=2 contraction is zero-padded to K=128 so the HAM clock-gate
         sees real array activity and un-throttles the PE to 2.4 GHz in
         time for the main matmuls (plus 2 filler matmuls bridge the gap).
         Then 8 main matmuls acc[i,(c,j)] += A_k^T @ [Bw|BwY0|BwY1]_k.
  ACT  : exp. B-side: 8 per-chunk calls with the per-partition bias AP
         -(X0^2+X1^2)/2 (the w factor rides the exp as ln w; the bias is
         chunk-dependent so per-chunk calls are forced). A-side: 2 calls of
         512, zero bias, interleaved [B0..B3 A0 B4 B5 A1 B6 B7] so psi and
         the mains are fed as early as possible.
  DVE  : one fused custom op for the bias tile ((Src0^2+Src1^2)*imm2), then
         the psi muls as fused (k,c)-broadcast TENSOR_TENSORs in stripes
         [1,1,2,2,1,1] (fine at both ends of the pipeline), then the
         PSUM->SBUF epilogue casts. GpSimd is kept IDLE through the body:
         DVE tensor_tensor's second read port is the DVE/GpSimd *shared*
         SBUF port, and a concurrent Pool op blocks it for the whole
         instruction (this serialized the v1 psi muls at ~2.6 cyc/elem).

The exp bias-as-argument trick keeps all x^2 terms out of the stationary (no
host-side arithmetic - host prep stays layout-only) and off the DVE.

The framework const memsets are stripped and the act-table load hoisted ahead
of the scalar queue's input-DMA wait so the measured window opens on the
first real compute op. Measured: 16.0-16.1us vs the 18.1us v1 baseline; the
window is body (~6.6us) + a fixed ~9.4us NRT tail (every sem in the file is
zeroed one EVENT_SEMAPHORE at a time, 51/engine, Tensor at ~127ns each -
runtime-injected, not controllable from the NEFF).
"""

import numpy as np
from contextlib import ExitStack

N_AXIS = 128          # grid points per axis
NPTS = 8192           # data points
NCORES = 8
XPC = NPTS // NCORES   # 1024 data points per core
NCHUNK = XPC // 128    # 8 contraction chunks of 128
GRID_LO, GRID_HI = -2.0, 2.0
MAX_SEM = 32           # walrus semaphore allocation cap

_CACHE = {}


def _register_sqsum():
    """Register a fused -(a^2+b^2)*C2 custom DVE op (idempotent)."""
    from concourse import dve_ops
    from concourse.dve_spec import Spec, Src0, Src1, C2, sq, lower
    from concourse.dve_uop import DveOpSpec

    name = "TENSOR_SQSUM_SCALE_X"
    for op in dve_ops.OPS:
        if op.name == name:
            return op
    spec = Spec(
        body=(sq(Src0) + sq(Src1)) * C2,
        reference=lambda in0, in1, s0, s1, imm2: (
            in0.astype(np.float32) ** 2 + in1.astype(np.float32) ** 2) * imm2,
    )
    opcode = max(dve_ops._SUB_OPCODE_FOR_NAME.values()) + 1
    assert opcode < 0x20
    dve_ops._SUB_OPCODE_FOR_NAME[name] = opcode
    shas = {}
    for ver in ("v3", "v4"):
        s = DveOpSpec(name=name, opcode=opcode, uops=lower(spec, ver=ver), rd1_en=True)
        shas[ver] = s.sha(ver)
    op = dve_ops.DveOp(name, spec, subdim=False, uops_sha=shas)
    dve_ops.OPS.append(op)
    dve_ops.CUSTOM_DVE_SPECS[name] = spec
    return op


def _patch_walrus_flags():
    """Cap the compiler's semaphore allocation (idempotent)."""
    import concourse.bass_utils as bu

    if getattr(bu.run_command, "_sem_cap_patched", False):
        return
    orig = bu.run_command

    def run_command_capped(argv, **kwargs):
        if argv and "walrus_driver" in str(argv[0]) and any(
                str(a).startswith("--neff-output-filename") for a in argv):
            argv = list(argv) + [f"--max-sem-num={MAX_SEM}"]
        return orig(argv, **kwargs)

    run_command_capped._sem_cap_patched = True
    bu.run_command = run_command_capped


def _hoist_act_table_load(nc):
    """Move the framework-inserted InstLoadActFuncSet ahead of the scalar
    queue's input-DMA wait so the ~1.3us table load overlaps the DMA instead
    of delaying the first exp behind it."""
    act_engine = nc.scalar.engine
    for b in nc.m.functions[0].blocks:
        insts = b.instructions
        load_idx = None
        for i, inst in enumerate(insts):
            if type(inst).__name__ == "InstLoadActFuncSet":
                load_idx = i
                break
        if load_idx is None:
            continue
        anchor = None
        for i in range(load_idx - 1, -1, -1):
            inst = insts[i]
            if getattr(inst, "engine", None) == act_engine:
                if type(inst).__name__ == "InstEventSemaphore":
                    anchor = i
                else:
                    break
        if anchor is not None:
            load = insts[load_idx]
            b.instructions.remove(load)
            b.instructions.insert(anchor, load)
        return


def _strip_const_memsets(nc):
    """Drop the Bass-preamble const-AP memsets (const-float32-0.0 etc.) when
    nothing references them. They are the first compute-class instructions in
    the NEFF and needlessly extend the measured execution window."""
    fn = nc.m.functions[0]
    drop = []
    for b in fn.blocks:
        for inst in b.instructions:
            if type(inst).__name__ == "InstMemset" and "memref='const-" in str(
                    inst.outs[0]):
                drop.append((b, inst))
    drop_set = {id(i) for _, i in drop}
    for b in fn.blocks:
        for inst in b.instructions:
            if id(inst) in drop_set:
                continue
            assert "const-" not in str(inst.ins), (
                f"const AP still referenced by {type(inst).__name__}: {inst.ins}")
    for b, inst in drop:
        b.instructions.remove(inst)


def _build_program():
    import concourse.bacc as bacc
    import concourse.mybir as mybir
    import concourse.tile as tile

    _patch_walrus_flags()
    sqsum = _register_sqsum()

    f32 = mybir.dt.float32
    f16 = mybir.dt.float16
    nc = bacc.Bacc("TRN2", target_bir_lowering=False, debug=False, num_devices=NCORES,
                   enable_partition_id=False, monotonic_sem_count=0)

    # Inputs:
    #   stat [128, 2176] f16: the args-matmul operands as 17 col-blocks of
    #                       128. Row 0: A blocks X0_k at cols 128k, B blocks
    #                       X1_k at 1024+128k, g at cols 2048:2176. Row 1:
    #                       ones under the X blocks, -g^2/2 under g. Rows
    #                       2:128 are ZERO: the K=2 contraction is padded to
    #                       K=128 so the args matmuls light up the whole PE
    #                       array and the HAM clock-gate sees real activity
    #                       (K=2 matmuls leave the array 98% idle and the PE
    #                       never un-throttles to 2.4 GHz for the mains).
    #   xp   [128, 18] f32: cols 0:8 = X0 chunks (partition = x in chunk),
    #                       cols 8:16 = X1 chunks, col 16 = 0.0 (exp bias).
    #   yc   [128, 16] f16: col 2k+c = Y_c chunk k.
    stat = nc.dram_tensor("stat", [128, 2176], f16, kind="ExternalInput")
    xp = nc.dram_tensor("xp", [128, 18], f32, kind="ExternalInput")
    yc = nc.dram_tensor("yc", [128, 16], f16, kind="ExternalInput")
    out = nc.dram_tensor("out", [128, 3 * N_AXIS], f16, kind="ExternalOutput")

    with tile.TileContext(nc) as tc, ExitStack() as ctx:
        singles = ctx.enter_context(tc.tile_pool(name="singles", bufs=1))
        psum = ctx.enter_context(tc.tile_pool(name="psum", bufs=1, space="PSUM"))

        s_stat = singles.tile([128, 2176], f16, tag="stat")
        s_xp = singles.tile([128, 18], f32, tag="xp")
        s_yc = singles.tile([128, 16], f16, tag="yc")
        nc.sync.dma_start(s_stat[:, :], stat[:, :])
        nc.sync.dma_start(s_xp[:, :], xp[:, :], single_packet=True)
        nc.sync.dma_start(s_yc[:, :], yc[:, :], single_packet=True)

        s_bias = singles.tile([128, NCHUNK], f32, tag="bias")
        s_a = singles.tile([128, NCHUNK * 128], f16, tag="a")
        s_rhs = singles.tile([128, NCHUNK, 3 * 128], f16, tag="rhs")
        s_out = singles.tile([128, 3 * N_AXIS], f16, tag="outt")

        # PSUM (bank-quantized): acc + 4 B-pair tiles (fine-grained exp deps)
        # + 2 A tiles + 1 warmup scratch = 8 banks.
        p_acc = psum.tile([128, 512], f32, tag="acc")
        p_b = [psum.tile([128, 256], f32, tag=f"b{i}", name=f"p_b{i}")
               for i in range(4)]
        p_a = [psum.tile([128, 512], f32, tag=f"a{i}", name=f"p_a{i}")
               for i in range(2)]
        p_scr = psum.tile([128, 512], f32, tag="scr")

        g_rhs = s_stat[:, 2048:2176]

        # Per-chunk exp bias: -(X0^2 + X1^2)/2, one fused DVE op.
        nc.vector._custom_dve(
            sqsum, out=s_bias[:, :],
            in0=s_xp[:, 0:NCHUNK], in1=s_xp[:, NCHUNK:2 * NCHUNK], imm2=-0.5)

        # args matmuls, K=2: arg[x, j] = X[x]*g_j - g_j^2/2. Per-block lhsT
        # pairs [X_chunk; ones]; the LDWEIGHTS stream pipelines under the
        # matmuls via the PE reorder window. B side first (psi hangs off it).
        for k in range(NCHUNK):
            nc.tensor.matmul(
                p_b[k // 2][:, 128 * (k % 2):128 * (k % 2 + 1)],
                s_stat[:, 1024 + 128 * k:1024 + 128 * (k + 1)],
                g_rhs, start=True, stop=True,
            )
        for k in range(NCHUNK):
            nc.tensor.matmul(
                p_a[k // 4][:, 128 * (k % 4):128 * (k % 4 + 1)],
                s_stat[:, 128 * k:128 * (k + 1)],
                g_rhs, start=True, stop=True,
            )
        # PE warmup fillers: keep the PE-busy window alive between the args
        # sweep and the psi-gated main matmuls so HAM un-throttles.
        for _ in range(2):
            nc.tensor.matmul(p_scr[:, 0:128], s_stat[:, 0:128], g_rhs,
                             start=True, stop=True)

        # exp: per-chunk B calls (chunk-dependent per-partition bias), A in
        # 512-wide calls placed where the main-matmul pipeline needs them.
        def b_exp(k):
            nc.scalar.activation(
                s_rhs[:, k, 0:128],
                p_b[k // 2][:, 128 * (k % 2):128 * (k % 2 + 1)],
                mybir.ActivationFunctionType.Exp,
                scale=1.0, bias=s_bias[:, k:k + 1],
            )

        def a_exp(h):
            nc.scalar.activation(
                s_a[:, 512 * h:512 * (h + 1)],
                p_a[h][:, :],
                mybir.ActivationFunctionType.Exp,
                scale=1.0, bias=s_xp[:, 16:17],
            )

        for k in (0, 1, 2, 3):
            b_exp(k)
        a_exp(0)
        for k in (4, 5):
            b_exp(k)
        a_exp(1)
        for k in (6, 7):
            b_exp(k)

        # psi: both products per stripe in one fused (k, c)-broadcast op.
        # Asymmetric stripes keep the last exp -> psi -> matmul chain short.
        def psi(k0, kw):
            ks = slice(k0, k0 + kw)
            nc.vector.tensor_tensor(
                s_rhs[:, ks, 128:384].rearrange("p k (c j) -> p k c j", j=128),
                s_rhs[:, ks, 0:128].unsqueeze(2)
                    .broadcast_to([128, kw, 2, 128]),
                s_yc[:, 2 * k0:2 * (k0 + kw)].rearrange("p (k c) -> p k c", c=2)
                    .unsqueeze(3).broadcast_to([128, kw, 2, 128]),
                mybir.AluOpType.mult,
            )

        stripes = [(0, 1), (1, 1), (2, 2), (4, 2), (6, 1), (7, 1)]
        for k0, kw in stripes:
            psi(k0, kw)

        # main matmuls: acc[i, (c, j)] = sum_x A_k[x, i] * rhs_k[x, (c, j)].
        for k in range(NCHUNK):
            nc.tensor.matmul(
                p_acc[:, 0:384],
                s_a[:, 128 * k:128 * (k + 1)],
                s_rhs[:, k, :],
                start=(k == 0), stop=(k == NCHUNK - 1),
            )

        # Epilogue: PSUM -> SBUF fp16 on Vector (the ACT copy was observed to
        # eat a ~0.5us PE->ACT sem hop), store split across the two HWDGE
        # queues as soon as each half's cast lands.
        nc.vector.tensor_copy(s_out[:, 0:192], p_acc[:, 0:192])
        nc.vector.tensor_copy(s_out[:, 192:384], p_acc[:, 192:384])
        nc.sync.dma_start(out[:, 0:192], s_out[:, 0:192], single_packet=True)
        nc.scalar.dma_start(out[:, 192:384], s_out[:, 192:384], single_packet=True)

    _strip_const_memsets(nc)
    nc.finalize()
    _hoist_act_table_load(nc)
    return nc


def _get_program():
    if "nc" not in _CACHE:
        _CACHE["nc"] = _build_program()
    return _CACHE["nc"]


def _host_inputs(X, Y):
    """Build the per-core input maps (layout prep only)."""
    X = np.ascontiguousarray(np.asarray(X, dtype=np.float32))
    Y = np.ascontiguousarray(np.asarray(Y, dtype=np.float32))
    gr = np.linspace(GRID_LO, GRID_HI, N_AXIS, dtype=np.float32)
    g2 = (-0.5 * gr * gr).astype(np.float32)

    in_maps = []
    for m in range(NCORES):
        sl = slice(m * XPC, (m + 1) * XPC)
        x0 = X[sl, 0].reshape(NCHUNK, 128)
        x1 = X[sl, 1].reshape(NCHUNK, 128)
        statm = np.zeros((128, 2176), np.float16)
        statm[1, 0:2048] = 1.0
        statm[0, 0:1024] = x0.ravel()        # A-side lhsT X rows
        statm[0, 1024:2048] = x1.ravel()     # B-side lhsT X rows
        statm[0, 2048:2176] = gr
        statm[1, 2048:2176] = g2
        xpm = np.zeros((128, 18), np.float32)
        xpm[:, 0:NCHUNK] = x0.T
        xpm[:, NCHUNK:2 * NCHUNK] = x1.T
        ycm = np.empty((128, 16), np.float16)
        ycm[:, 0:16:2] = Y[sl, 0].reshape(NCHUNK, 128).T
        ycm[:, 1:16:2] = Y[sl, 1].reshape(NCHUNK, 128).T
        in_maps.append({"stat": statm, "xp": xpm, "yc": ycm})
    return in_maps


def run_on_cores(X, Y, **spmd_kwargs):
    """Run the SPMD kernel; returns BassKernelResults."""
    from concourse.bass_utils import run_bass_kernel_spmd

    nc = _get_program()
    in_maps = _host_inputs(X, Y)
    res = run_bass_kernel_spmd(nc, in_maps, core_ids=list(range(NCORES)),
                               **spmd_kwargs)
    return res


def kernel(X, Y):
    res = run_on_cores(X, Y)
    # Sum the per-core partial feature maps, then normalize.
    acc = np.zeros((128, 3 * N_AXIS), np.float64)
    for r in res.results:
        acc += r["out"]
    fm = acc.reshape(128, 3, N_AXIS)                 # [i, c, j]
    full = fm.transpose(1, 2, 0).astype(np.float32)  # [c, j, i]
    full[1] /= full[0]
    full[2] /= full[0]
    return np.ascontiguousarray(full)


# revision 33
# speedup vs baseline: 1.0126x; 1.0126x over previous
"""ConvCNP encoder kernel for 8x TRN2 NeuronCores.

Math: the reference computes, for a 128x128 uniform grid g=(xs[i], ys[j]) and
n=8192 data points X (2-D) with values psi(Y) = [1, Y0, Y1]:

    Gram[g, x] = exp(-0.5*||g - X[x]||^2)
    fm = Gram @ psi                  # (G, 3); column 0 == row-sum (denominator)
    out[c, j, i] = fm[(i, j), c], with c=1,2 normalized by column 0.

The squared distance is separable over the grid axes (xs == ys == the same
128-point linspace g), and each factor splits into PE-friendly terms:

    Gram[(i,j), x] = A[i, x] * Bw[j, x]
      A[i, x]  = exp(g_i*X0[x] - g_i^2/2)
      Bw[j, x] = exp(g_j*X1[x] - g_j^2/2 - (X0[x]^2 + X1[x]^2)/2)

so, with Bc = Bw * psi_c (row-wise):  fm[(i,j), c] = sum_x Bc[j, x] * A[i, x].

Sharding: the DATA-POINT axis x across the 8 cores - 1024 points per core
(8 chunks of 128), grid replicated. Each core computes the partial
(un-normalized) feature map for the FULL grid over its x-slice; the host sums
the 8 partials and normalizes.

Engine split per core (the big idea vs the v1 kernel: NO elementwise
squared-distance pass at all - the exp arguments are tiny matmuls):

  PE   : 16 args-matmuls, one per (side, chunk): lhsT = [X block; ones]
         column-pair from the stat tile, rhs = the shared [g; -g^2/2] pair
         -> arg[x, j] = X[x]*g_j - g_j^2/2 in per-pair PSUM tiles (separate
         tiles so each exp's semaphore gates on ITS matmuls, not all 16).
         The K=2 contraction is zero-padded to K=128 so the HAM clock-gate
         sees real array activity and un-throttles the PE to 2.4 GHz in
         time for the main matmuls (plus 2 filler matmuls bridge the gap).
         Then 8 main matmuls acc[i,(c,j)] += A_k^T @ [Bw|BwY0|BwY1]_k.
  ACT  : exp. B-side: 8 per-chunk calls with the per-partition bias AP
         -(X0^2+X1^2)/2 (the w factor rides the exp as ln w; the bias is
         chunk-dependent so per-chunk calls are forced). A-side: 2 calls of
         512, zero bias, interleaved [B0..B3 A0 B4 B5 A1 B6 B7] so psi and
         the mains are fed as early as possible.
  DVE  : one fused custom op for the bias tile ((Src0^2+Src1^2)*imm2), then
         the psi muls as fused (k,c)-broadcast TENSOR_TENSORs in stripes
         [1,1,2,2,1,1] (fine at both ends of the pipeline), then the
         PSUM->SBUF epilogue casts. GpSimd is kept IDLE through the body:
         DVE tensor_tensor's second read port is the DVE/GpSimd *shared*
         SBUF port, and a concurrent Pool op blocks it for the whole
         instruction (this serialized the v1 psi muls at ~2.6 cyc/elem).

The exp bias-as-argument trick keeps all x^2 terms out of the stationary (no
host-side arithmetic - host prep stays layout-only) and off the DVE.

The framework const memsets are stripped and the act-table load hoisted ahead
of the scalar queue's input-DMA wait so the measured window opens on the
first real compute op. Measured: 16.0-16.1us vs the 18.1us v1 baseline; the
window is body (~6.6us) + a fixed ~9.4us NRT tail (every sem in the file is
zeroed one EVENT_SEMAPHORE at a time, 51/engine, Tensor at ~127ns each -
runtime-injected, not controllable from the NEFF).
"""

import numpy as np
from contextlib import ExitStack

N_AXIS = 128          # grid points per axis
NPTS = 8192           # data points
NCORES = 8
XPC = NPTS // NCORES   # 1024 data points per core
NCHUNK = XPC // 128    # 8 contraction chunks of 128
GRID_LO, GRID_HI = -2.0, 2.0
MAX_SEM = 32           # walrus semaphore allocation cap

_CACHE = {}


def _register_sqsum():
    """Register a fused -(a^2+b^2)*C2 custom DVE op (idempotent)."""
    from concourse import dve_ops
    from concourse.dve_spec import Spec, Src0, Src1, C2, sq, lower
    from concourse.dve_uop import DveOpSpec

    name = "TENSOR_SQSUM_SCALE_X"
    for op in dve_ops.OPS:
        if op.name == name:
            return op
    spec = Spec(
        body=(sq(Src0) + sq(Src1)) * C2,
        reference=lambda in0, in1, s0, s1, imm2: (
            in0.astype(np.float32) ** 2 + in1.astype(np.float32) ** 2) * imm2,
    )
    opcode = max(dve_ops._SUB_OPCODE_FOR_NAME.values()) + 1
    assert opcode < 0x20
    dve_ops._SUB_OPCODE_FOR_NAME[name] = opcode
    shas = {}
    for ver in ("v3", "v4"):
        s = DveOpSpec(name=name, opcode=opcode, uops=lower(spec, ver=ver), rd1_en=True)
        shas[ver] = s.sha(ver)
    op = dve_ops.DveOp(name, spec, subdim=False, uops_sha=shas)
    dve_ops.OPS.append(op)
    dve_ops.CUSTOM_DVE_SPECS[name] = spec
    return op


def _patch_walrus_flags():
    """Cap the compiler's semaphore allocation (idempotent)."""
    import concourse.bass_utils as bu

    if getattr(bu.run_command, "_sem_cap_patched", False):
        return
    orig = bu.run_command

    def run_command_capped(argv, **kwargs):
        if argv and "walrus_driver" in str(argv[0]) and any(
                str(a).startswith("--neff-output-filename") for a in argv):
            argv = list(argv) + [f"--max-sem-num={MAX_SEM}"]
        return orig(argv, **kwargs)

    run_command_capped._sem_cap_patched = True
    bu.run_command = run_command_capped


def _hoist_act_table_load(nc):
    """Move the framework-inserted InstLoadActFuncSet ahead of the scalar
    queue's input-DMA wait so the ~1.3us table load overlaps the DMA instead
    of delaying the first exp behind it."""
    act_engine = nc.scalar.engine
    for b in nc.m.functions[0].blocks:
        insts = b.instructions
        load_idx = None
        for i, inst in enumerate(insts):
            if type(inst).__name__ == "InstLoadActFuncSet":
                load_idx = i
                break
        if load_idx is None:
            continue
        anchor = None
        for i in range(load_idx - 1, -1, -1):
            inst = insts[i]
            if getattr(inst, "engine", None) == act_engine:
                if type(inst).__name__ == "InstEventSemaphore":
                    anchor = i
                else:
                    break
        if anchor is not None:
            load = insts[load_idx]
            b.instructions.remove(load)
            b.instructions.insert(anchor, load)
        return


def _strip_const_memsets(nc):
    """Drop the Bass-preamble const-AP memsets (const-float32-0.0 etc.) when
    nothing references them. They are the first compute-class instructions in
    the NEFF and needlessly extend the measured execution window."""
    fn = nc.m.functions[0]
    drop = []
    for b in fn.blocks:
        for inst in b.instructions:
            if type(inst).__name__ == "InstMemset" and "memref='const-" in str(
                    inst.outs[0]):
                drop.append((b, inst))
    drop_set = {id(i) for _, i in drop}
    for b in fn.blocks:
        for inst in b.instructions:
            if id(inst) in drop_set:
                continue
            assert "const-" not in str(inst.ins), (
                f"const AP still referenced by {type(inst).__name__}: {inst.ins}")
    for b, inst in drop:
        b.instructions.remove(inst)


def _build_program():
    import concourse.bacc as bacc
    import concourse.mybir as mybir
    import concourse.tile as tile

    _patch_walrus_flags()
    sqsum = _register_sqsum()

    f32 = mybir.dt.float32
    f16 = mybir.dt.float16
    nc = bacc.Bacc("TRN2", target_bir_lowering=False, debug=False, num_devices=NCORES,
                   enable_partition_id=False, monotonic_sem_count=0)

    # Inputs:
    #   stat [128, 2176] f16: the args-matmul operands as 17 col-blocks of
    #                       128. Row 0: A blocks X0_k at cols 128k, B blocks
    #                       X1_k at 1024+128k, g at cols 2048:2176. Row 1:
    #                       ones under the X blocks, -g^2/2 under g. Rows
    #                       2:128 are ZERO: the K=2 contraction is padded to
    #                       K=128 so the args matmuls light up the whole PE
    #                       array and the HAM clock-gate sees real activity
    #                       (K=2 matmuls leave the array 98% idle and the PE
    #                       never un-throttles to 2.4 GHz for the mains).
    #   xp   [128, 18] f32: cols 0:8 = X0 chunks (partition = x in chunk),
    #                       cols 8:16 = X1 chunks, col 16 = 0.0 (exp bias).
    #   yc   [128, 16] f16: col 2k+c = Y_c chunk k.
    stat = nc.dram_tensor("stat", [128, 2176], f16, kind="ExternalInput")
    xp = nc.dram_tensor("xp", [128, 18], f32, kind="ExternalInput")
    yc = nc.dram_tensor("yc", [128, 16], f16, kind="ExternalInput")
    out = nc.dram_tensor("out", [128, 3 * N_AXIS], f16, kind="ExternalOutput")

    with tile.TileContext(nc) as tc, ExitStack() as ctx:
        singles = ctx.enter_context(tc.tile_pool(name="singles", bufs=1))
        psum = ctx.enter_context(tc.tile_pool(name="psum", bufs=1, space="PSUM"))

        s_stat = singles.tile([128, 2176], f16, tag="stat")
        s_xp = singles.tile([128, 18], f32, tag="xp")
        s_yc = singles.tile([128, 16], f16, tag="yc")
        nc.sync.dma_start(s_stat[:, :], stat[:, :])
        nc.sync.dma_start(s_xp[:, :], xp[:, :], single_packet=True)
        nc.sync.dma_start(s_yc[:, :], yc[:, :], single_packet=True)

        s_bias = singles.tile([128, NCHUNK], f32, tag="bias")
        s_a = singles.tile([128, NCHUNK * 128], f16, tag="a")
        s_rhs = singles.tile([128, NCHUNK, 3 * 128], f16, tag="rhs")
        s_out = singles.tile([128, 3 * N_AXIS], f16, tag="outt")

        # PSUM (bank-quantized): acc + 4 B-pair tiles (fine-grained exp deps)
        # + 2 A tiles + 1 warmup scratch = 8 banks.
        p_acc = psum.tile([128, 512], f32, tag="acc")
        p_b = [psum.tile([128, 256], f32, tag=f"b{i}", name=f"p_b{i}")
               for i in range(4)]
        p_a = [psum.tile([128, 512], f32, tag=f"a{i}", name=f"p_a{i}")
               for i in range(2)]
        p_scr = psum.tile([128, 512], f32, tag="scr")

        g_rhs = s_stat[:, 2048:2176]

        # Per-chunk exp bias: -(X0^2 + X1^2)/2, one fused DVE op.
        nc.vector._custom_dve(
            sqsum, out=s_bias[:, :],
            in0=s_xp[:, 0:NCHUNK], in1=s_xp[:, NCHUNK:2 * NCHUNK], imm2=-0.5)

        # args matmuls, K=2: arg[x, j] = X[x]*g_j - g_j^2/2. Per-block lhsT
        # pairs [X_chunk; ones]; the LDWEIGHTS stream pipelines under the
        # matmuls via the PE reorder window. B side first (psi hangs off it).
        for k in range(NCHUNK):
            nc.tensor.matmul(
                p_b[k // 2][:, 128 * (k % 2):128 * (k % 2 + 1)],
                s_stat[:, 1024 + 128 * k:1024 + 128 * (k + 1)],
                g_rhs, start=True, stop=True,
            )
        for k in range(NCHUNK):
            nc.tensor.matmul(
                p_a[k // 4][:, 128 * (k % 4):128 * (k % 4 + 1)],
                s_stat[:, 128 * k:128 * (k + 1)],
                g_rhs, start=True, stop=True,
            )
        # PE warmup fillers: keep the PE-busy window alive between the args
        # sweep and the psi-gated main matmuls so HAM un-throttles.
        for _ in range(4):
            nc.tensor.matmul(p_scr[:, 0:128], s_stat[:, 0:128], g_rhs,
                             start=True, stop=True)

        # exp: per-chunk B calls (chunk-dependent per-partition bias), A in
        # 512-wide calls placed where the main-matmul pipeline needs them.
        def b_exp(k):
            nc.scalar.activation(
                s_rhs[:, k, 0:128],
                p_b[k // 2][:, 128 * (k % 2):128 * (k % 2 + 1)],
                mybir.ActivationFunctionType.Exp,
                scale=1.0, bias=s_bias[:, k:k + 1],
            )

        def a_exp(h):
            nc.scalar.activation(
                s_a[:, 512 * h:512 * (h + 1)],
                p_a[h][:, :],
                mybir.ActivationFunctionType.Exp,
                scale=1.0, bias=s_xp[:, 16:17],
            )

        for k in (0, 1, 2, 3):
            b_exp(k)
        a_exp(0)
        for k in (4, 5):
            b_exp(k)
        a_exp(1)
        for k in (6, 7):
            b_exp(k)

        # psi: both products per stripe in one fused (k, c)-broadcast op.
        # Asymmetric stripes keep the last exp -> psi -> matmul chain short.
        def psi(k0, kw):
            ks = slice(k0, k0 + kw)
            nc.vector.tensor_tensor(
                s_rhs[:, ks, 128:384].rearrange("p k (c j) -> p k c j", j=128),
                s_rhs[:, ks, 0:128].unsqueeze(2)
                    .broadcast_to([128, kw, 2, 128]),
                s_yc[:, 2 * k0:2 * (k0 + kw)].rearrange("p (k c) -> p k c", c=2)
                    .unsqueeze(3).broadcast_to([128, kw, 2, 128]),
                mybir.AluOpType.mult,
            )

        stripes = [(0, 1), (1, 1), (2, 2), (4, 2), (6, 1), (7, 1)]
        for k0, kw in stripes:
            psi(k0, kw)

        # main matmuls: acc[i, (c, j)] = sum_x A_k[x, i] * rhs_k[x, (c, j)].
        for k in range(NCHUNK):
            nc.tensor.matmul(
                p_acc[:, 0:384],
                s_a[:, 128 * k:128 * (k + 1)],
                s_rhs[:, k, :],
                start=(k == 0), stop=(k == NCHUNK - 1),
            )

        # Epilogue: PSUM -> SBUF fp16 on Vector (the ACT copy was observed to
        # eat a ~0.5us PE->ACT sem hop), store split across the two HWDGE
        # queues as soon as each half's cast lands.
        nc.vector.tensor_copy(s_out[:, 0:192], p_acc[:, 0:192])
        nc.vector.tensor_copy(s_out[:, 192:384], p_acc[:, 192:384])
        nc.sync.dma_start(out[:, 0:192], s_out[:, 0:192], single_packet=True)
        nc.scalar.dma_start(out[:, 192:384], s_out[:, 192:384], single_packet=True)

    _strip_const_memsets(nc)
    nc.finalize()
    _hoist_act_table_load(nc)
    return nc


def _get_program():
    if "nc" not in _CACHE:
        _CACHE["nc"] = _build_program()
    return _CACHE["nc"]


def _host_inputs(X, Y):
    """Build the per-core input maps (layout prep only)."""
    X = np.ascontiguousarray(np.asarray(X, dtype=np.float32))
    Y = np.ascontiguousarray(np.asarray(Y, dtype=np.float32))
    gr = np.linspace(GRID_LO, GRID_HI, N_AXIS, dtype=np.float32)
    g2 = (-0.5 * gr * gr).astype(np.float32)

    in_maps = []
    for m in range(NCORES):
        sl = slice(m * XPC, (m + 1) * XPC)
        x0 = X[sl, 0].reshape(NCHUNK, 128)
        x1 = X[sl, 1].reshape(NCHUNK, 128)
        statm = np.zeros((128, 2176), np.float16)
        statm[1, 0:2048] = 1.0
        statm[0, 0:1024] = x0.ravel()        # A-side lhsT X rows
        statm[0, 1024:2048] = x1.ravel()     # B-side lhsT X rows
        statm[0, 2048:2176] = gr
        statm[1, 2048:2176] = g2
        xpm = np.zeros((128, 18), np.float32)
        xpm[:, 0:NCHUNK] = x0.T
        xpm[:, NCHUNK:2 * NCHUNK] = x1.T
        ycm = np.empty((128, 16), np.float16)
        ycm[:, 0:16:2] = Y[sl, 0].reshape(NCHUNK, 128).T
        ycm[:, 1:16:2] = Y[sl, 1].reshape(NCHUNK, 128).T
        in_maps.append({"stat": statm, "xp": xpm, "yc": ycm})
    return in_maps


def run_on_cores(X, Y, **spmd_kwargs):
    """Run the SPMD kernel; returns BassKernelResults."""
    from concourse.bass_utils import run_bass_kernel_spmd

    nc = _get_program()
    in_maps = _host_inputs(X, Y)
    res = run_bass_kernel_spmd(nc, in_maps, core_ids=list(range(NCORES)),
                               **spmd_kwargs)
    return res


def kernel(X, Y):
    res = run_on_cores(X, Y)
    # Sum the per-core partial feature maps, then normalize.
    acc = np.zeros((128, 3 * N_AXIS), np.float64)
    for r in res.results:
        acc += r["out"]
    fm = acc.reshape(128, 3, N_AXIS)                 # [i, c, j]
    full = fm.transpose(1, 2, 0).astype(np.float32)  # [c, j, i]
    full[1] /= full[0]
    full[2] /= full[0]
    return np.ascontiguousarray(full)
